# revision 23
# baseline (speedup 1.0000x reference)
"""Trainium2 Bass kernel for SoftMoE (LayerNorm + cosine routing + per-expert MLP).

Sharding: pure data-parallel over batch B=8 -> one batch element per NeuronCore.
No collectives. Each core computes its full (N, D) output slice.

fp8 DoubleRow edition: the three 17-GFLOP GEMMs (logits, dispatch, combine)
run as fp8e4 DoubleRow matmuls (two 128-row contraction subtiles per pass).
exp(logits) clusters at 1.0 (cosine logits are ~N(0, 0.02)) where e4m3 steps
are ~9%, so quantizing E directly erases the routing signal. The kernel
centers instead: v = E-1 is what gets quantized (full resolution on the
variation) and the rank-1 parts are restored exactly in higher precision:

  dispatch: x_ln^T @ E = s ox 1  + x_ln^T @ v,  s_d = sum_n x_ln[n,d]
            (s via FD=1 matmuls on bf16 x_n * tinv during the LN phase,
             added per-partition at the PSUM eviction)
  combine:  E @ so = 1 ox S + v @ so,           S_d = sum_es so[es,d]
            (S via M=1 ones-matmuls on bf16 so; restored with a rank-1
             ones_row ox S_row matmul accumulated into the same PSUM)

The MLP (8.6 GFLOP) stays bf16. All quantization scales are powers of two
(x_ln*16, x_n*256, mu*256, v*1024, so*2048), folded exactly into activation
scales. sd comes free from the exp eviction's accum_out; sc comes from a
DoubleRow ones-column matmul on the centered combine weights.
"""

import numpy as np
from contextlib import ExitStack

import concourse.bass as bass
import concourse.tile as tile
from concourse import bacc
from concourse import mybir
from concourse.masks import make_identity

FP32 = mybir.dt.float32
BF16 = mybir.dt.bfloat16
FP8 = mybir.dt.float8e4
AF = mybir.ActivationFunctionType
ALU = mybir.AluOpType
AX = mybir.AxisListType
DR = mybir.MatmulPerfMode.DoubleRow

P = 128
LN_EPS = 1e-5

# quantization scales (powers of two; descales folded into activation scales)
S_XL = 16.0        # x_ln -> fp8
S_XN = 256.0       # x_n (unit rows) -> fp8
S_MU = 256.0       # raw mu -> fp8
S_V = 1024.0       # (E - 1) -> fp8
S_SO = 2048.0      # slot_out -> fp8

# CoreSim doesn't implement Gelu; dev_sim flips this to validate the pipeline
# with Tanh standing in for Gelu. Hardware builds keep the real Gelu.
SIM_SAFE_GELU = False


def _bcast_ap(handle, p, free):
    """AP reading a 1-D DRAM tensor broadcast across p partitions."""
    return bass.AP(tensor=handle, offset=0, ap=[[0, p], [1, free]])


def build_softmoe(N, D, E, S, H, *, apply_gamma_beta=True, apply_b1=True,
                  apply_b2=True):
    assert S == P
    ES = E * S
    NT, KD, NE, QH = N // P, D // P, ES // P, H // P
    CN = min(512, N); JN = N // CN       # n-chunks
    CE = min(512, ES); JE = ES // CE     # es-chunks
    CD = min(512, D); JD = D // CD       # d-chunks
    EPC = CE // P                        # experts per es-chunk
    NP_, DP_, EP_ = NT // 2, KD // 2, NE // 2   # DoubleRow pair counts

    nc = bacc.Bacc(None, target_bir_lowering=False, debug=False)

    x_h = nc.dram_tensor("x", [N, D], FP32, kind="ExternalInput")
    g_h = nc.dram_tensor("gamma", [D], FP32, kind="ExternalInput")
    be_h = nc.dram_tensor("beta", [D], FP32, kind="ExternalInput")
    mu_h = nc.dram_tensor("mu", [D, E, S], FP32, kind="ExternalInput")
    sc_h = nc.dram_tensor("scale", [1], FP32, kind="ExternalInput")
    w1_h = nc.dram_tensor("W1", [E, D, H], FP32, kind="ExternalInput")
    b1_h = nc.dram_tensor("b1", [E, H], FP32, kind="ExternalInput")
    w2_h = nc.dram_tensor("W2", [E, H, D], FP32, kind="ExternalInput")
    b2_h = nc.dram_tensor("b2", [E, D], FP32, kind="ExternalInput")
    out_h = nc.dram_tensor("out", [N, D], FP32, kind="ExternalOutput")

    # scratch split per chunk so phase N+1 can start on chunk 0 while
    # phase N still writes chunk 3 (DRAM deps are per-tensor)
    xn_d = [nc.dram_tensor(f"xn_scr{j}", [CN, D], BF16) for j in range(JN)]
    et_d = [nc.dram_tensor(f"et_scr{c}", [CE, N], BF16) for c in range(JE)]
    et8_d = nc.dram_tensor("et8_scr", [ES, N], FP8)

    with tile.TileContext(nc, pool_alloc_mode="queue") as tc, ExitStack() as ctx:
        small = ctx.enter_context(tc.tile_pool(name="small", bufs=1))
        # One PSUM pool for the whole kernel: matmul tiles share the "mmps"
        # tag (5 banks), PE-transpose tiles share "pst" (2 banks), and the
        # long-lived P1 column accumulators (n2 norms + s_d) get a dedicated
        # full bank ("accps") so overlapping-phase matmul tiles can never
        # rotate onto their bank mid-accumulation.
        psum = ctx.enter_context(tc.tile_pool(name="psum", bufs=5, space="PSUM"))

        ones_b = small.tile([P, 1], BF16, tag="ones_b")
        nc.vector.memset(ones_b, 1.0)
        ones8 = small.tile([P, 2, 16], FP8, tag="ones8")
        nc.vector.memset(ones8, 1.0)
        ones_row = small.tile([1, P], BF16, tag="ones_row")
        nc.vector.memset(ones_row, 1.0)
        s_bc = small.tile([P, 1], FP32, tag="s_bc")
        nc.gpsimd.dma_start(out=s_bc, in_=_bcast_ap(sc_h, P, 1))
        tinv = small.tile([P, NT], FP32, tag="tinv")
        tinv_b = small.tile([P, NT], BF16, tag="tinv_b")
        minv = small.tile([P, NE], FP32, tag="minv")
        minvq = small.tile([P, NE], FP32, tag="minvq")
        sd = small.tile([P, NE], FP32, tag="sd")
        sdinv = small.tile([P, NE], FP32, tag="sdinv")
        s_sb = small.tile([P, KD], FP32, tag="s_sb")
        S_row = small.tile([1, D], BF16, tag="S_row")
        ident_b = small.tile([P, P], BF16, tag="ident_b")
        make_identity(nc, ident_b)
        if apply_b1:
            ident_f = small.tile([P, P], FP32, tag="ident_f")
            make_identity(nc, ident_f)
        if apply_gamma_beta:
            gm_bc = small.tile([P, D], FP32, tag="gm_bc")
            nc.gpsimd.dma_start(out=gm_bc, in_=_bcast_ap(g_h, P, D))
            bt_bc = small.tile([P, D], FP32, tag="bt_bc")
            nc.gpsimd.dma_start(out=bt_bc, in_=_bcast_ap(be_h, P, D))

        # persistent fp8 operand tiles (DoubleRow pair layout [P, 2, free])
        xl8p = ctx.enter_context(tc.tile_pool(name="xl8_pool", bufs=1))
        xl8 = [xl8p.tile([P, 2, D], FP8, tag=f"xl8{t}", name=f"xl8{t}")
               for t in range(NP_)]
        so8p = ctx.enter_context(tc.tile_pool(name="so8_pool", bufs=1))
        so8 = [so8p.tile([P, 2, D], FP8, tag=f"so8{t}", name=f"so8{t}")
               for t in range(EP_)]

        # ------------- P2a: mu load/cast + column norms (overlaps P1) -------
        mu8_ctx = ExitStack()
        mu8p = mu8_ctx.enter_context(tc.tile_pool(name="mu8_pool", bufs=1))
        mu8 = [mu8p.tile([P, 2, ES], FP8, tag=f"mu8{q}", name=f"mu8{q}")
               for q in range(DP_)]

        # ------------- P1: LayerNorm + x_n (bf16) + x_ln (fp8) --------------
        # Fast path (gamma==1, beta==0, scale>0): LN rstd cancels against the
        # l2 norm: x_n = (x - mean) * c with c = s/sqrt(D*var) and
        # tinv = sqrt(D*var/(var+eps))/s, so x_n*tinv == x_ln exactly.
        # s_d = sum_n x_ln[n, d] accumulates via FD=1 matmuls (PE is idle
        # here); x_ln*16 goes to fp8 pair tiles for the dispatch GEMM.
        sinv_bc = small.tile([P, 1], FP32, tag="sinv_bc")
        nc.vector.reciprocal(out=sinv_bc[:], in_=s_bc[:])
        with tc.tile_pool(name="p1", bufs=4) as p1, \
                tc.tile_pool(name="p1s", bufs=8) as p1s, \
                tc.tile_pool(name="p2a", bufs=3) as p2a:
            # Column-wise accumulation groups share one bank. A start=True
            # matmul zeroes the whole 2KB zero-region (the full bank), wiping
            # sibling columns' partials — so zero the bank once via memset and
            # run every column matmul with start=False.
            acc_ps = psum.tile([P, 512], FP32, tag="accps", bufs=1,
                               name="acc_ps")
            nc.vector.memset(acc_ps, 0.0)
            n2 = acc_ps[:, 0:NE]
            s_ps = acc_ps[:, 32:32 + KD]
            for i in range(NT):
                # front-load mu (two tiles per x tile) so mu8 completes by
                # i=7 and the logits phase can overlap the P1 tail
                for k in (2 * i, 2 * i + 1):
                    if k >= KD:
                        continue
                    muf = p2a.tile([P, ES], BF16, tag="muf")
                    nc.gpsimd.dma_start(
                        out=muf[:],
                        in_=mu_h[k * P:(k + 1) * P].rearrange("p e s -> p (e s)"))
                    nc.scalar.activation(out=mu8[k // 2][:, k % 2, :],
                                         in_=muf[:], func=AF.Copy, scale=S_MU)
                    msq = p2a.tile([P, ES], BF16, tag="msq")
                    nc.scalar.activation(out=msq[:], in_=muf[:],
                                         func=AF.Square)
                    for e in range(NE):
                        nc.tensor.matmul(n2[:, e:e + 1],
                                         msq[:, e * P:(e + 1) * P], ones_b[:],
                                         start=False, stop=(k == KD - 1),
                                         skip_group_check=True)
                xf = p1.tile([P, D], FP32, tag="xf")
                xeng = nc.sync if i % 2 == 0 else nc.scalar
                xeng.dma_start(out=xf[:], in_=x_h[i * P:(i + 1) * P, :])
                sub = min(512, D)
                nsub = D // sub
                st = p1s.tile([P, nsub, 6], FP32, tag="st")
                for u in range(nsub):
                    nc.vector.bn_stats(out=st[:, u, :],
                                       in_=xf[:, u * sub:(u + 1) * sub])
                mv = p1s.tile([P, 2], FP32, tag="mv")
                nc.vector.bn_aggr(out=mv[:], in_=st[:])
                xnb = p1.tile([P, D], BF16, tag="xnb")
                if not apply_gamma_beta:
                    den = p1s.tile([P, 1], FP32, tag="den")
                    nc.vector.tensor_scalar_add(den[:], mv[:, 1:2], LN_EPS)
                    rden = p1s.tile([P, 1], FP32, tag="rden")
                    nc.vector.reciprocal(out=rden[:], in_=den[:])
                    w_ = p1s.tile([P, 1], FP32, tag="w_")
                    nc.vector.tensor_mul(w_[:], mv[:, 1:2], rden[:])
                    sq1 = p1s.tile([P, 1], FP32, tag="sq1")
                    nc.scalar.activation(out=sq1[:], in_=mv[:, 1:2],
                                         func=AF.Sqrt, scale=float(D))
                    rc = p1s.tile([P, 1], FP32, tag="rc")
                    nc.vector.reciprocal(out=rc[:], in_=sq1[:])
                    c_ = p1s.tile([P, 1], FP32, tag="c_")
                    nc.vector.tensor_scalar_mul(c_[:], rc[:], s_bc[:])
                    sq2 = p1s.tile([P, 1], FP32, tag="sq2")
                    nc.scalar.activation(out=sq2[:], in_=w_[:], func=AF.Sqrt,
                                         scale=float(D))
                    nc.vector.tensor_scalar_mul(tinv[:, i:i + 1], sq2[:],
                                                sinv_bc[:])
                    nc.vector.tensor_scalar(out=xnb[:], in0=xf[:],
                                            scalar1=mv[:, 0:1], scalar2=c_[:],
                                            op0=ALU.subtract, op1=ALU.mult)
                else:
                    lv = p1s.tile([P, 1], FP32, tag="lv")
                    nc.vector.tensor_scalar_add(lv[:], mv[:, 1:2], LN_EPS)
                    q_ = p1s.tile([P, 1], FP32, tag="q_")
                    nc.scalar.activation(out=q_[:], in_=lv[:], func=AF.Sqrt)
                    r = p1s.tile([P, 1], FP32, tag="r")
                    nc.vector.reciprocal(out=r[:], in_=q_[:])
                    xln = p1.tile([P, D], FP32, tag="xln")
                    nc.vector.tensor_scalar(out=xln[:], in0=xf[:],
                                            scalar1=mv[:, 0:1], scalar2=r[:],
                                            op0=ALU.subtract, op1=ALU.mult)
                    nc.vector.tensor_mul(xln[:], xln[:], gm_bc[:])
                    nc.vector.tensor_add(xln[:], xln[:], bt_bc[:])
                    sq = p1.tile([P, D], FP32, tag="sq")
                    nc.vector.tensor_mul(sq[:], xln[:], xln[:])
                    ss = p1s.tile([P, 1], FP32, tag="ss")
                    nc.vector.tensor_reduce(out=ss[:], in_=sq[:], axis=AX.X,
                                            op=ALU.add)
                    qs = p1s.tile([P, 1], FP32, tag="qs")
                    nc.scalar.activation(out=qs[:], in_=ss[:], func=AF.Sqrt)
                    u_ = p1s.tile([P, 1], FP32, tag="u_")
                    nc.vector.reciprocal(out=u_[:], in_=qs[:])
                    t_ = p1s.tile([P, 1], FP32, tag="t_")
                    nc.vector.tensor_scalar_mul(t_[:], u_[:], s_bc[:])
                    nc.vector.reciprocal(out=tinv[:, i:i + 1], in_=t_[:])
                    nc.vector.tensor_scalar_mul(xnb[:], xln[:], t_[:])
                # x_ln in fp8 (scale 16) for the dispatch GEMM lhsT
                ts16 = p1s.tile([P, 1], FP32, tag="ts16")
                nc.vector.tensor_scalar_mul(ts16[:], tinv[:, i:i + 1], S_XL)
                nc.scalar.activation(out=xl8[i // 2][:, i % 2, :], in_=xnb[:],
                                     func=AF.Copy, scale=ts16[:])
                nc.vector.tensor_copy(out=tinv_b[:, i:i + 1],
                                      in_=tinv[:, i:i + 1])
                # s_d accumulation: sum_n xnb * tinv == sum_n x_ln
                for dsl in range(KD):
                    nc.tensor.matmul(s_ps[:, dsl:dsl + 1],
                                     xnb[:, dsl * P:(dsl + 1) * P],
                                     tinv_b[:, i:i + 1],
                                     start=False, stop=(i == NT - 1),
                                     skip_group_check=True)
                nc.gpsimd.dma_start(out=xn_d[i // 4][(i % 4) * P:
                                                     (i % 4 + 1) * P, :],
                                    in_=xnb[:])
            sqn = small.tile([P, NE], FP32, tag="sqn")
            nc.scalar.activation(out=sqn[:], in_=n2[:], func=AF.Sqrt)
            nc.vector.reciprocal(out=minv[:], in_=sqn[:])
            nc.vector.tensor_scalar_mul(minvq[:], minv[:],
                                        1.0 / (S_XN * S_MU))
            nc.vector.tensor_copy(out=s_sb[:], in_=s_ps[:])

        # ------------- P2b: logits^T + exp (fp8 DoubleRow) ------------------
        # et_d holds centered (E-1)*1024 bf16 (for dispatch transposes);
        # et8_d holds the same values in fp8 (combine lhsT).
        sdall = small.tile([P, NE * JN], FP32, tag="sdall")
        with tc.tile_pool(name="xnT_pool", bufs=2) as xntp, \
                tc.tile_pool(name="x8_pool", bufs=1) as x8p, \
                tc.tile_pool(name="p2b", bufs=6) as p2b:
            xnT8 = [x8p.tile([P, DP_, 2, CN], FP8, tag=f"xnT8{j}",
                             name=f"xnT8{j}") for j in range(JN)]
            for j in range(JN):
                xnt3 = xntp.tile([P, KD, CN], BF16, tag="xnt3",
                                 name=f"xnt3{j}")
                for k in range(KD):
                    eng = nc.sync if k % 2 == 0 else nc.scalar
                    eng.dma_start(
                        out=xnt3[:, k, :],
                        in_=xn_d[j][:, k * P:(k + 1) * P],
                        transpose=True)
                    nc.vector.tensor_scalar_mul(xnT8[j][:, k // 2, k % 2, :],
                                                xnt3[:, k, :], S_XN)
            # hybrid order: n-chunk 0 for all experts first (overlaps the P1
            # tail), then expert-outer so et_d[c] chunks complete early and
            # the dispatch phase can overlap the logits tail
            orders = [(0, e) for e in range(NE)] + \
                     [(j, e) for e in range(NE) for j in range(1, JN)]
            for (j, e) in orders:
                    ps = psum.tile([P, CN], FP32, tag="mmps",
                                   name=f"lgps{e}_{j}")
                    for q in range(DP_):
                        nc.tensor.matmul(ps[:],
                                         mu8[q][:, :, e * P:(e + 1) * P],
                                         xnT8[j][:, q, :, :],
                                         start=(q == 0), stop=(q == DP_ - 1),
                                         perf_mode=DR)
                    ett = p2b.tile([P, CN], BF16, tag="ett")
                    nc.scalar.activation(out=ett[:], in_=ps[:], func=AF.Exp,
                                         scale=minvq[:, e:e + 1],
                                         accum_out=sdall[:, e * JN + j:
                                                         e * JN + j + 1])
                    cet = p2b.tile([P, CN], BF16, tag="cet")
                    nc.vector.tensor_scalar(out=cet[:], in0=ett[:],
                                            scalar1=S_V, scalar2=S_V,
                                            op0=ALU.mult, op1=ALU.subtract)
                    et8t = p2b.tile([P, CN], FP8, tag="et8t")
                    nc.vector.tensor_scalar(out=et8t[:], in0=ett[:],
                                            scalar1=S_V, scalar2=S_V,
                                            op0=ALU.mult, op1=ALU.subtract)
                    nc.scalar.dma_start(
                        out=et_d[e // EPC][(e % EPC) * P:(e % EPC + 1) * P,
                                           j * CN:(j + 1) * CN],
                        in_=cet[:])
                    nc.scalar.dma_start(
                        out=et8_d[e * P:(e + 1) * P, j * CN:(j + 1) * CN],
                        in_=et8t[:])
            for e in range(NE):
                nc.vector.tensor_reduce(
                    out=sd[:, e:e + 1],
                    in_=sdall[:, e * JN:(e + 1) * JN], axis=AX.X, op=ALU.add)
            nc.vector.reciprocal(out=sdinv[:], in_=sd[:])
        mu8_ctx.close()  # release mu8 pool before the dispatch/MLP phase

        # ------------- P3: dispatch (fp8 DR) + per-expert MLP (bf16) --------
        accS = psum.tile([P, CD], FP32, tag="accps", bufs=1, name="accS")
        nc.vector.memset(accS, 0.0)
        p3_ctx = ExitStack()
        sitp = p3_ctx.enter_context(tc.tile_pool(name="sit_pool", bufs=1))
        echp = p3_ctx.enter_context(tc.tile_pool(name="ech", bufs=2))
        vchp = p3_ctx.enter_context(tc.tile_pool(name="vch", bufs=2))
        mlp = p3_ctx.enter_context(tc.tile_pool(name="mlp", bufs=8))
        mlpw2 = p3_ctx.enter_context(tc.tile_pool(name="mlp_w2", bufs=4))
        mlpsm = p3_ctx.enter_context(tc.tile_pool(name="mlp_sm", bufs=4))
        sobp = p3_ctx.enter_context(tc.tile_pool(name="sob", bufs=4))
        siT = [sitp.tile([P, CE], BF16, tag=f"siT{d}", name=f"siT{d}")
               for d in range(KD)]
        gelu_f = AF.Tanh if SIM_SAFE_GELU else AF.Gelu
        for c in range(JE):
            ech = echp.tile([P, NT, CE], BF16, tag="ech")
            vech8 = vchp.tile([P, NP_, 2, CE], FP8, tag="vech8")
            for k in range(NT):
                nc.sync.dma_start(
                    out=ech[:, k, :],
                    in_=et_d[c][:, k * P:(k + 1) * P],
                    transpose=True)
                nc.vector.tensor_copy(out=vech8[:, k // 2, k % 2, :],
                                      in_=ech[:, k, :])
            for dsl in range(KD):
                ps = psum.tile([P, CE], FP32, tag="mmps", name=f"sips{c}_{dsl}")
                for t in range(NP_):
                    nc.tensor.matmul(ps[:],
                                     xl8[t][:, :, dsl * P:(dsl + 1) * P],
                                     vech8[:, t, :, :],
                                     start=(t == 0), stop=(t == NP_ - 1),
                                     perf_mode=DR)
                # siT = psum / (S_XL*S_V) + s_d  (rank-1 restore, bf16)
                nc.scalar.activation(out=siT[dsl][:], in_=ps[:],
                                     func=AF.Identity,
                                     scale=1.0 / (S_XL * S_V),
                                     bias=s_sb[:, dsl:dsl + 1])
            # MLP for the experts covered by this es-chunk
            sob_chunk = []
            for e in range(c * EPC, (c + 1) * EPC):
                le = e - c * EPC  # expert offset within chunk columns
                psh = psum.tile([P, H], FP32, tag="mmps", name=f"psh{e}")
                for k in range(KD):
                    w1b = mlp.tile([P, H], BF16, tag="w1b", bufs=16)
                    nc.gpsimd.dma_start(out=w1b[:],
                                        in_=w1_h[e, k * P:(k + 1) * P, :])
                    nc.tensor.matmul(psh[:],
                                     siT[k][:, le * P:(le + 1) * P],
                                     w1b[:], start=(k == 0),
                                     stop=(k == KD - 1 and not apply_b1))
                if apply_b1:
                    # psh += outer(sd_e, b1_e); gelu scale then yields
                    # gelu(sdinv*raw + b1)
                    pst0 = psum.tile([P, P], FP32, tag="pst", name=f"psdr{e}",
                                     bufs=2)
                    nc.tensor.transpose(pst0[:1, :], sd[:, e:e + 1],
                                        ident_f[:])
                    sdrow = mlpsm.tile([1, P], BF16, tag="sdrow")
                    nc.vector.tensor_copy(out=sdrow[:], in_=pst0[:1, :])
                    b1row = mlpsm.tile([1, H], BF16, tag="b1row")
                    nc.gpsimd.dma_start(out=b1row[:], in_=b1_h[e:e + 1, :])
                    nc.tensor.matmul(psh[:], sdrow[:], b1row[:],
                                     start=False, stop=True)
                hbf = mlp.tile([P, H], BF16, tag="hbf", bufs=2)
                nc.scalar.activation(out=hbf[:], in_=psh[:], func=gelu_f,
                                     scale=sdinv[:, e:e + 1])
                hT = mlp.tile([P, QH, P], BF16, tag="hT", bufs=2)
                for q in range(QH):
                    pst = psum.tile([P, P], BF16, tag="pst",
                                    name=f"pst{e}_{q}", bufs=2)
                    nc.tensor.transpose(pst[:], hbf[:, q * P:(q + 1) * P],
                                        ident_b[:])
                    nc.vector.tensor_copy(out=hT[:, q, :], in_=pst[:])
                w2q = [mlpw2.tile([P, D], BF16, tag="w2q", bufs=6,
                                  name=f"w2q{e}_{q}") for q in range(QH)]
                for q in range(QH):
                    nc.gpsimd.dma_start(out=w2q[q][:],
                                        in_=w2_h[e, q * P:(q + 1) * P, :])
                if apply_b2:
                    b2row = mlpsm.tile([1, D], BF16, tag="b2row")
                    nc.gpsimd.dma_start(out=b2row[:], in_=b2_h[e:e + 1, :])
                so_bf = sobp.tile([P, D], BF16, tag="so_bf",
                                  name=f"so_bf{e}")
                sob_chunk.append(so_bf)
                for dch in range(JD):
                    pso = psum.tile([P, CD], FP32, tag="mmps",
                                    name=f"pso{e}_{dch}")
                    for q in range(QH):
                        nc.tensor.matmul(
                            pso[:], hT[:, q, :],
                            w2q[q][:, dch * CD:(dch + 1) * CD],
                            start=(q == 0),
                            stop=(q == QH - 1 and not apply_b2))
                    if apply_b2:
                        nc.tensor.matmul(
                            pso[:], ones_row[:],
                            b2row[:, dch * CD:(dch + 1) * CD],
                            start=False, stop=True)
                    nc.scalar.activation(
                        out=so_bf[:, dch * CD:(dch + 1) * CD], in_=pso[:],
                        func=AF.Copy)
                    nc.vector.tensor_scalar_mul(
                        so8[e // 2][:, e % 2, dch * CD:(dch + 1) * CD],
                        pso[:], S_SO)

            # S contributions for this chunk's experts, batched here so the
            # matmuls' waits on the scalar so_bf evictions don't stall the
            # strict-FIFO PE stream mid-chunk. Accumulated across experts in
            # a dedicated PSUM bank (memset + start=False: a start=True
            # would zero the sibling d-chunk rows in the same bank).
            for le in range(EPC):
                e = c * EPC + le
                for dch in range(JD):
                    nc.tensor.matmul(accS[32 * dch:32 * dch + 1, :],
                                     ones_b[:],
                                     sob_chunk[le][:, dch * CD:(dch + 1) * CD],
                                     start=False, stop=(e == E - 1),
                                     skip_group_check=True,
                                     tile_position=(0, 32 * dch))
        p3_ctx.close()  # release dispatch/MLP pools; keep so8 for P4
        # S_row = 2^21 * S (the combine PSUM carries S_V*S_SO = 2^21)
        for dch in range(JD):
            nc.scalar.activation(out=S_row[:, dch * CD:(dch + 1) * CD],
                                 in_=accS[32 * dch:32 * dch + 1, :],
                                 func=AF.Copy, scale=S_V * S_SO)

        # ------------- P4: combine (fp8 DR + rank-1 restore) ----------------
        with tc.tile_pool(name="p4", bufs=2) as p4, \
                tc.tile_pool(name="p4s", bufs=4) as p4s:
            for i in range(NT):
                etb = p4.tile([P, EP_, 2, P], FP8, tag="etb")
                for pr in range(EP_):
                    eng = nc.sync if pr % 2 == 0 else nc.scalar
                    eng.dma_start(
                        out=etb[:, pr, :, :],
                        in_=et8_d[pr * 256:(pr + 1) * 256,
                                  i * P:(i + 1) * P].rearrange(
                                      "(two p) n -> p two n", two=2))
                pso_ = [psum.tile([P, CD], FP32, tag="mmps",
                                  name=f"ops{i}_{j}") for j in range(JD)]
                pssc = psum.tile([P, 1], FP32, tag="pst", name=f"pssc{i}",
                                 bufs=2)
                for pr in range(EP_):
                    for dch in range(JD):
                        nc.tensor.matmul(
                            pso_[dch][:], etb[:, pr, :, :],
                            so8[pr][:, :, dch * CD:(dch + 1) * CD],
                            start=(pr == 0), stop=False,
                            perf_mode=DR)
                    nc.tensor.matmul(pssc[:], etb[:, pr, :, :],
                                     ones8[:, :, 0:1],
                                     start=(pr == 0), stop=(pr == EP_ - 1),
                                     perf_mode=DR)
                for dch in range(JD):
                    nc.tensor.matmul(pso_[dch][:], ones_row[:],
                                     S_row[:, dch * CD:(dch + 1) * CD],
                                     start=False, stop=True)
                # out = psum / (2^21 * sc),  sc = 2048 + pssc/1024
                qd = p4s.tile([P, 1], FP32, tag="qd")
                nc.vector.tensor_scalar(out=qd[:], in0=pssc[:],
                                        scalar1=S_SO,
                                        scalar2=float(ES) * S_V * S_SO,
                                        op0=ALU.mult, op1=ALU.add)
                scinv = p4s.tile([P, 1], FP32, tag="scinv")
                nc.vector.reciprocal(out=scinv[:], in_=qd[:])
                outt = p4.tile([P, D], FP32, tag="outt")
                for dch in range(JD):
                    nc.scalar.activation(
                        out=outt[:, dch * CD:(dch + 1) * CD],
                        in_=pso_[dch][:], func=AF.Copy, scale=scinv[:])
                nc.sync.dma_start(out=out_h[i * P:(i + 1) * P, :],
                                  in_=outt[:])
    nc.compile()
    return nc


_NC_CACHE = {}


def _get_nc(N, D, E, S, H, flags):
    key = (N, D, E, S, H, flags)
    if key not in _NC_CACHE:
        _NC_CACHE[key] = build_softmoe(
            N, D, E, S, H, apply_gamma_beta=flags[0], apply_b1=flags[1],
            apply_b2=flags[2])
    return _NC_CACHE[key]


def kernel(x, gamma, beta, mu, scale, W1, b1, W2, b2):
    from concourse.bass_utils import run_bass_kernel_spmd

    x = np.ascontiguousarray(np.asarray(x, dtype=np.float32))
    gamma = np.ascontiguousarray(np.asarray(gamma, dtype=np.float32))
    beta = np.ascontiguousarray(np.asarray(beta, dtype=np.float32))
    mu = np.ascontiguousarray(np.asarray(mu, dtype=np.float32))
    scale = np.ascontiguousarray(np.asarray(scale, dtype=np.float32))
    W1 = np.ascontiguousarray(np.asarray(W1, dtype=np.float32))
    b1 = np.ascontiguousarray(np.asarray(b1, dtype=np.float32))
    W2 = np.ascontiguousarray(np.asarray(W2, dtype=np.float32))
    b2 = np.ascontiguousarray(np.asarray(b2, dtype=np.float32))

    B, N, D = x.shape
    _, E, S = mu.shape
    H = W1.shape[2]
    n_cores = 8
    assert B == n_cores, f"kernel hardcoded for B == {n_cores}, got {B}"

    flags = (
        # generic LN path also needed when scale <= 0 (fast path takes ln(s))
        bool(np.any(gamma != 1.0) or np.any(beta != 0.0)
             or np.any(scale <= 0.0)),
        bool(np.any(b1 != 0.0)),
        bool(np.any(b2 != 0.0)),
    )
    nc = _get_nc(N, D, E, S, H, flags)

    shared = dict(gamma=gamma, beta=beta, mu=mu, scale=scale, W1=W1, b1=b1,
                  W2=W2, b2=b2)
    in_maps = [dict(x=x[b], **shared) for b in range(n_cores)]
    import os
    trace = bool(os.environ.get("SOFTMOE_TRACE"))
    res = run_bass_kernel_spmd(nc, in_maps, core_ids=list(range(n_cores)),
                               trace=trace)
    global LAST_RESULT
    LAST_RESULT = res
    return np.stack([r["out"] for r in res.results], axis=0)


LAST_RESULT = None
